# revision 28
# baseline (speedup 1.0000x reference)
"""DistSAGE 3-layer GraphSAGE forward on 8 TRN2 NeuronCores (Bass/Tile).

Strategy (graph/data parallel, per the DistSAGE recipe):
  - Partition the 512 seed nodes across 8 cores (64 each, LPT-balanced by
    an additive 2-hop cost estimate); build per-core dependency-driven
    blocks on the host. No inter-core communication; weights replicated.
  - Layer 0 streams per-dst-tile dense bands (dst rows stored transposed
    + per-edge source rows, bf16, pre-interleaved for line-rate DMA).
    The S' aggregation masks are GENERATED ON-CHIP by the vector engine
    (one-hot: (colidx == dstcol[p]) * inv_deg[p]) from tiny per-chunk
    metadata, then used as the STATIONARY matmul operand with the 256-wide
    message groups streaming: mean0[d, f] += mask_k.T @ msgs_k.
  - No DRAM round-trip between layers: as each layer-0 output tile is
    ReLU'd in SBUF, small fanout matmuls scatter-accumulate its
    contribution into layer-1 mean accumulators held in PSUM
    (meanT1[f, d1] += o2_chunk.T @ M1_tile). Extra (non-l1) nodes are
    grouped by primary layer-1 dst half so most fanouts are narrow; the
    l1_out-region tiles (and multi-half nodes) use full-width host-baked
    masks. Layers 1/2 then finish with a ~10us tail (h1/h2 stay in SBUF).
"""

import heapq

import numpy as np

P = 128
NCORES = 8
NUM_DST = (61952, 5632, 512)
FEAT = 256
OUTW = (256, 256, 19)
SEEDS_PER_CORE = NUM_DST[2] // NCORES  # 64
N1_TILES = 6  # layer-1 dst tiles (n1 <= 768 on every core; asserted)
DHALF = N1_TILES * P // 2  # 384
PAD_DST = 200.0  # one-hot "never matches" sentinel column index


def _bf16():
    import ml_dtypes

    return ml_dtypes.bfloat16


# ---------------------------------------------------------------------------
# Host-side block construction
# ---------------------------------------------------------------------------


def _balance(ids, deg, n_buckets):
    """LPT bin-packing: reorder ids so consecutive 128-groups have ~equal
    total degree (only full 128-groups are balanced)."""
    if n_buckets <= 1 or len(ids) < n_buckets * P:
        return ids
    order = np.argsort(-deg[ids], kind="stable")
    heap = [(0.0, b, 0) for b in range(n_buckets)]
    heapq.heapify(heap)
    buckets = [[] for _ in range(n_buckets)]
    for i in order:
        load, b, cnt = heapq.heappop(heap)
        buckets[b].append(ids[i])
        cnt += 1
        if cnt < P:
            heapq.heappush(heap, (load + deg[ids[i]], b, cnt))
    return np.concatenate([np.asarray(b, dtype=ids.dtype) for b in buckets])


def _seed_partition(esrc0, edst0, esrc1, edst1, esrc2, edst2, deg0, deg1):
    """LPT-balance seeds across cores by an additive 2-hop cost estimate."""
    h = np.zeros(NUM_DST[1], np.float64)
    np.add.at(h, edst1, deg0[esrc1].astype(np.float64))
    cost = np.zeros(NUM_DST[2], np.float64)
    np.add.at(cost, edst2, h[esrc2] + deg1[esrc2].astype(np.float64))
    order = np.argsort(-cost, kind="stable")
    heap = [(0.0, cc, 0) for cc in range(NCORES)]
    heapq.heapify(heap)
    groups = [[] for _ in range(NCORES)]
    for s in order:
        load, cc, cnt = heapq.heappop(heap)
        groups[cc].append(s)
        cnt += 1
        if cnt < SEEDS_PER_CORE:
            heapq.heappush(heap, (load + cost[s], cc, cnt))
    return [np.array(g, dtype=np.int64) for g in groups]


def _block_for_core(seeds, esrc0, edst0, esrc1, edst1, esrc2, edst2,
                    deg0, deg1):
    """Raw per-core block: l1_out, l0 extras split by primary l1 d-half,
    and the edge lists (in global node ids / l1 positions)."""
    pos2 = np.full(NUM_DST[2], -1, np.int32)
    pos2[seeds] = np.arange(SEEDS_PER_CORE, dtype=np.int32)
    sel2 = pos2[edst2] >= 0
    es2, ed2g = esrc2[sel2], edst2[sel2]
    l1_extra = np.setdiff1d(np.unique(es2), seeds)
    l1_out = np.concatenate([seeds, l1_extra])
    n1 = len(l1_out)
    assert n1 <= N1_TILES * P

    pos1 = np.full(NUM_DST[1], -1, np.int32)
    pos1[l1_out] = np.arange(n1, dtype=np.int32)
    sel1 = pos1[edst1] >= 0
    es1, ed1g = esrc1[sel1], edst1[sel1]
    ed1 = pos1[ed1g].astype(np.int64)  # l1 positions [0, n1)
    inv1 = (1.0 / np.maximum(deg1[ed1g], 1.0)).astype(np.float32)

    l0_extra = np.setdiff1d(np.unique(es1), l1_out)

    # primary d-half per extra node: halves touched by its l1 edges
    emask = np.zeros((NUM_DST[0], 2), bool)
    np.logical_or.at(emask, (es1, np.minimum(ed1 // DHALF, 1)), True)
    m0 = emask[l0_extra, 0]
    m1 = emask[l0_extra, 1]
    g_both = l0_extra[m0 & m1]
    g0 = l0_extra[m0 & ~m1]
    g1 = l0_extra[~m0 & m1]

    ed2 = pos2[ed2g].astype(np.int64)
    inv2 = (1.0 / np.maximum(deg2_of(ed2g, edst2), 1.0)).astype(np.float32)
    es2l = pos1[es2].astype(np.int64)

    return dict(
        seeds=seeds, l1_out=l1_out, n1=n1,
        g0=g0, g1=g1, g_both=g_both,
        es1=es1, ed1=ed1, inv1=inv1,
        es2l=es2l, ed2=ed2, inv2=inv2,
    )


_DEG2 = None


def deg2_of(ids, edst2):
    global _DEG2
    if _DEG2 is None:
        _DEG2 = np.bincount(edst2, minlength=NUM_DST[2]).astype(np.float32)
    return _DEG2[ids]


def build_host(inputs):
    global _DEG2
    _DEG2 = None
    esrc0 = np.asarray(inputs["esrc0"]).astype(np.int64)
    edst0 = np.asarray(inputs["edst0"]).astype(np.int64)
    esrc1 = np.asarray(inputs["esrc1"]).astype(np.int64)
    edst1 = np.asarray(inputs["edst1"]).astype(np.int64)
    esrc2 = np.asarray(inputs["esrc2"]).astype(np.int64)
    edst2 = np.asarray(inputs["edst2"]).astype(np.int64)
    x = np.asarray(inputs["x"], dtype=np.float32)

    deg0 = np.bincount(edst0, minlength=NUM_DST[0]).astype(np.float32)
    deg1 = np.bincount(edst1, minlength=NUM_DST[1]).astype(np.float32)

    seed_groups = _seed_partition(esrc0, edst0, esrc1, edst1, esrc2, edst2,
                                  deg0, deg1)
    blocks = [
        _block_for_core(seed_groups[c], esrc0, edst0, esrc1, edst1, esrc2,
                        edst2, deg0, deg1)
        for c in range(NCORES)
    ]

    # ---- uniform group sizes (padded to max over cores, then to 128) ----
    def padlen(key):
        m = max(len(b[key]) for b in blocks)
        return -(-max(m, 1) // P) * P if m > 0 else 0

    L0, L1, LB = padlen("g0"), padlen("g1"), padlen("g_both")
    NL1 = N1_TILES * P  # 768 rows for the l1_out region

    # row layout in l0_out: [l1_out(768) | g0(L0) | g1(L1) | both(LB)]
    # per-core l0 node list (global ids), padded with dummy = l1_out[0]
    l0_rows = []
    def balanced(ids, deg):
        nfull = (len(ids) // P) * P
        if nfull < P:
            return ids
        return np.concatenate([_balance(ids[:nfull], deg, nfull // P),
                               ids[nfull:]])

    for b in blocks:
        dummy = b["l1_out"][0]
        g0b = balanced(b["g0"], deg0) if L0 else np.zeros(0, np.int64)
        g0v = np.full(L0, dummy, np.int64)
        g0v[: len(g0b)] = g0b
        g1b = balanced(b["g1"], deg0) if L1 else np.zeros(0, np.int64)
        g1v = np.full(L1, dummy, np.int64)
        g1v[: len(g1b)] = g1b
        gbv = np.full(LB, dummy, np.int64)
        gbv[: len(b["g_both"])] = b["g_both"]
        l1v = np.full(NL1, dummy, np.int64)
        l1v[: b["n1"]] = b["l1_out"]
        l0_rows.append(np.concatenate([l1v, g0v, g1v, gbv]))

    n0_pad = NL1 + L0 + L1 + LB
    T0 = n0_pad // P

    # tile processing order: narrow g0 tiles, narrow g1 tiles, full tiles
    # (l1_out region first 6 + both-group tiles last)
    t_g0 = list(range(N1_TILES, N1_TILES + L0 // P))
    t_g1 = list(range(N1_TILES + L0 // P, N1_TILES + (L0 + L1) // P))
    t_l1 = list(range(N1_TILES))
    t_b = list(range(N1_TILES + (L0 + L1) // P, T0))
    tile_order = t_g0 + t_g1 + t_l1 + t_b
    tile_kind = {}  # tile -> ("narrow", g) | ("full", None)
    for t in t_g0:
        tile_kind[t] = ("narrow", 0)
    for t in t_g1:
        tile_kind[t] = ("narrow", 1)
    for t in t_l1 + t_b:
        tile_kind[t] = ("full", None)

    # ---- layer-0 per-tile edge slots (per-edge; dedup not needed) ----
    # per core: positions of l0 rows for mapping layer-0 edges (first
    # occurrence wins; duplicate rows are dummy pads and receive no edges)
    pos0s = []
    for c in range(NCORES):
        rows = l0_rows[c]
        uniq, first_idx = np.unique(rows, return_index=True)
        pos0 = np.full(NUM_DST[0], -1, np.int64)
        pos0[uniq] = first_idx
        pos0s.append(pos0)

    # per-core, per-tile layer-0 edges: (x_row, dstcol, inv)
    tile_edges = [[None] * T0 for _ in range(NCORES)]
    for c, b in enumerate(blocks):
        pos0 = pos0s[c]
        sel0 = pos0[edst0] >= 0
        es0, ed0g = esrc0[sel0], edst0[sel0]
        ed0 = pos0[ed0g]
        inv0 = (1.0 / np.maximum(deg0[ed0g], 1.0)).astype(np.float32)
        tt = ed0 // P
        order = np.argsort(tt, kind="stable")
        es0, ed0, inv0, tt = es0[order], ed0[order], inv0[order], tt[order]
        starts = np.searchsorted(tt, np.arange(T0))
        ends = np.searchsorted(tt, np.arange(T0) + 1)
        for t in range(T0):
            s, e = starts[t], ends[t]
            tile_edges[c][t] = (es0[s:e], (ed0[s:e] - t * P), inv0[s:e])

    K = [
        max(1, max(-(-len(tile_edges[c][t][0]) // P) for c in range(NCORES)))
        for t in range(T0)
    ]
    Kmax = max(K)
    sp_off = np.concatenate([[0], np.cumsum(K)]).astype(np.int64)
    n_sp_cols = int(sp_off[-1])
    goff = np.concatenate([[0], np.cumsum([1 + k for k in K])]).astype(np.int64)
    n_groups = int(goff[-1])

    # ---- layer-1 fanout plans ----
    # narrow tiles: 3 base passes (+ uniform extra passes) of one-hot DVE gen
    # full tiles: host-baked dense [128, 768] masks
    # per core per tile: list of (row, dst_l1pos, val)
    l1fan = [[None] * T0 for _ in range(NCORES)]
    for c, b in enumerate(blocks):
        pos0 = pos0s[c]
        src_r = pos0[b["es1"]]
        assert (src_r >= 0).all()
        tt = src_r // P
        order = np.argsort(tt, kind="stable")
        sr, dd, vv, tt = (src_r[order], b["ed1"][order], b["inv1"][order],
                          tt[order])
        starts = np.searchsorted(tt, np.arange(T0))
        ends = np.searchsorted(tt, np.arange(T0) + 1)
        for t in range(T0):
            s, e = starts[t], ends[t]
            l1fan[c][t] = (sr[s:e] - t * P, dd[s:e], vv[s:e])

    # layer-0 agg chunk roles: first ndve chunks DVE-generated, rest DMA'd
    DVE_FRAC = 0.55
    ndve = [max(1, min(K[t], int(round(K[t] * DVE_FRAC)))) for t in range(T0)]
    ndma = [K[t] - ndve[t] for t in range(T0)]
    sp0_off = np.concatenate([[0], np.cumsum(ndma)]).astype(np.int64)
    n_sp0_cols = int(sp0_off[-1])
    n_narrow = sum(1 for t in tile_order if tile_kind[t][0] == "narrow")

    # fused band2 column layout (per processing-order tile)
    t_cols = {}
    for t in tile_order:
        kind, g = tile_kind[t]
        fw = DHALF if kind == "narrow" else N1_TILES * P
        t_cols[t] = FEAT * (1 + K[t]) + ndma[t] * P + fw
    boff = {}
    acc = 0
    for t in tile_order:
        boff[t] = acc
        acc += t_cols[t]
    n_band2_cols = acc

    bf16 = _bf16()
    x16 = x.astype(bf16)

    per_core = []
    for c in range(NCORES):
        b = blocks[c]
        band = np.zeros((P, n_groups, FEAT), bf16)
        meta_dst = np.full((P, n_sp_cols), PAD_DST, np.float32)
        meta_val = np.zeros((P, n_sp_cols), np.float32)
        for t in range(T0):
            g0c = int(goff[t])
            rows = l0_rows[c][t * P : (t + 1) * P]
            blk = np.ascontiguousarray(x16[rows])  # [128, 256]
            band[:, g0c, 0:P] = blk[:, 0:P].T
            band[:, g0c, P:FEAT] = blk[:, P:FEAT].T
            es, dc, iv = tile_edges[c][t]
            ne = len(es)
            for k in range(K[t]):
                a, e2 = k * P, min((k + 1) * P, ne)
                if a >= e2:
                    break
                band[: e2 - a, g0c + 1 + k, :] = x16[es[a:e2]]
                meta_dst[: e2 - a, int(sp_off[t]) + k] = dc[a:e2]
                meta_val[: e2 - a, int(sp_off[t]) + k] = iv[a:e2]

        # baked dense masks for the DMA-assigned layer-0 chunks
        sp0 = np.zeros((P, n_sp0_cols, P), np.float32)
        for t in range(T0):
            es, dc, iv = tile_edges[c][t]
            ne = len(es)
            for j in range(ndma[t]):
                k = ndve[t] + j
                a, e2 = k * P, min((k + 1) * P, ne)
                if a >= e2:
                    continue
                sp0[np.arange(e2 - a), int(sp0_off[t]) + j, dc[a:e2]] = iv[a:e2]

        # layer-1 fanout masks: narrow [128, 384] + full [128, 768], dense
        m1n, m1full = [], []
        for t in tile_order:
            kind, g = tile_kind[t]
            rows, dd, vv = l1fan[c][t]
            if kind == "narrow":
                W = np.zeros((P, DHALF), np.float32)
                np.add.at(W, (rows, dd - g * DHALF), vv)
                m1n.append(W.astype(bf16))
            else:
                W = np.zeros((P, N1_TILES * P), np.float32)
                np.add.at(W, (rows, dd), vv)
                m1full.append(W.astype(bf16))
        m1n = (np.concatenate(m1n, axis=1) if m1n
               else np.zeros((P, 0), bf16))
        m1full = (np.concatenate(m1full, axis=1) if m1full
                  else np.zeros((P, 0), bf16))

        # layer-2 fanout masks [6][128, 64]
        m2 = np.zeros((P, N1_TILES, SEEDS_PER_CORE), np.float32)
        u = b["es2l"] // P
        r2 = b["es2l"] % P
        np.add.at(m2, (r2, u, b["ed2"]), b["inv2"])

        # fused per-tile stripe: [hdT | msgs | dma-masks | fanout mask]
        sp016 = sp0.reshape(P, n_sp0_cols * P).astype(bf16)
        band2 = np.zeros((P, n_band2_cols), bf16)
        ni2 = fi2 = 0
        for t in tile_order:
            kind, g = tile_kind[t]
            bo = int(boff[t])
            nb = FEAT * (1 + K[t])
            band2[:, bo : bo + nb] = band.reshape(P, n_groups * FEAT)[
                :, int(goff[t]) * FEAT : int(goff[t]) * FEAT + nb]
            mo = bo + nb
            if ndma[t]:
                band2[:, mo : mo + ndma[t] * P] = sp016[
                    :, int(sp0_off[t]) * P : (int(sp0_off[t]) + ndma[t]) * P]
            fo = mo + ndma[t] * P
            if kind == "narrow":
                band2[:, fo : fo + DHALF] = m1n[
                    :, ni2 * DHALF : (ni2 + 1) * DHALF]
                ni2 += 1
            else:
                band2[:, fo : fo + N1_TILES * P] = m1full[
                    :, fi2 * N1_TILES * P : (fi2 + 1) * N1_TILES * P]
                fi2 += 1
        per_core.append(dict(
            band=np.ascontiguousarray(band.reshape(P, n_groups * FEAT)),
            band2=np.ascontiguousarray(band2),
            meta_dst=meta_dst,
            meta_val=meta_val,
            sp0=np.ascontiguousarray(sp016),
            m1n=np.ascontiguousarray(m1n),
            m1full=np.ascontiguousarray(m1full),
            m2=np.ascontiguousarray(
                m2.reshape(P, N1_TILES * SEEDS_PER_CORE).astype(bf16)
            ),
        ))

    n_full_tiles = sum(1 for t in tile_order if tile_kind[t][0] == "full")

    # start/stop flags for the mean1 accumulator chunks (keyed by d-half g)
    first_t = {0: None, 1: None}
    last_t = {0: None, 1: None}
    for t in tile_order:
        kind, g = tile_kind[t]
        gs = [g] if kind == "narrow" else [0, 1]
        for gg in gs:
            if first_t[gg] is None:
                first_t[gg] = t
            last_t[gg] = t

    return dict(
        blocks=blocks,
        T0=T0, K=K, Kmax=Kmax, sp_off=sp_off, n_sp_cols=n_sp_cols,
        goff=goff, n_groups=n_groups,
        tile_order=tile_order, tile_kind=tile_kind,
        ndve=ndve, ndma=ndma, sp0_off=sp0_off, n_sp0_cols=n_sp0_cols,
        n_narrow=n_narrow, boff=boff, t_cols=t_cols,
        n_band2_cols=n_band2_cols,
        n_full_tiles=n_full_tiles,
        first_t=first_t, last_t=last_t,
        per_core=per_core,
        n0_pad=n0_pad,
        weights=tuple(
            (
                np.asarray(inputs[f"W_self{l}"], np.float32),
                np.asarray(inputs[f"W_neigh{l}"], np.float32),
                np.asarray(inputs[f"b{l}"], np.float32),
            )
            for l in range(3)
        ),
    )


# ---------------------------------------------------------------------------
# Numpy simulation of the device kernel (validation aid; fp32 stand-in)
# ---------------------------------------------------------------------------


def simulate_core(meta, c, return_debug=False):
    pc = meta["per_core"][c]
    T0, K, sp_off, goff = meta["T0"], meta["K"], meta["sp_off"], meta["goff"]
    band = pc["band"].astype(np.float32).reshape(P, meta["n_groups"], FEAT)
    colidx = np.arange(P, dtype=np.float32)

    ws0, wn0, b0 = meta["weights"][0]
    ws1, wn1, b1 = meta["weights"][1]
    ws2, wn2, b2 = meta["weights"][2]

    mean1T = np.zeros((FEAT, N1_TILES * P), np.float32)
    h1 = np.zeros((N1_TILES * P, FEAT), np.float32)
    full_i = 0
    narrow_i = 0
    m1full = pc["m1full"].astype(np.float32)
    m1n = pc["m1n"].astype(np.float32)
    sp0 = pc["sp0"].astype(np.float32).reshape(P, -1, P)
    ndve, ndma, sp0_off = meta["ndve"], meta["ndma"], meta["sp0_off"]
    for t in meta["tile_order"]:
        kind, g = meta["tile_kind"][t]
        g0c = int(goff[t])
        hdT = np.concatenate(
            [band[:, g0c, 0:P], band[:, g0c, P:FEAT]], axis=1
        )  # [128f, 2*128d] halves
        mean0 = np.zeros((P, FEAT), np.float32)
        for k in range(K[t]):
            if k < ndve[t]:
                dst = pc["meta_dst"][:, int(sp_off[t]) + k].astype(np.float32)
                val = pc["meta_val"][:, int(sp_off[t]) + k].astype(np.float32)
                mask = (colidx[None, :] == dst[:, None]) * val[:, None]
            else:
                mask = sp0[:, int(sp0_off[t]) + (k - ndve[t]), :]
            mean0 += mask.T @ band[:, g0c + 1 + k, :]
        hd = np.concatenate([hdT[:, 0:P].T, hdT[:, P : 2 * P].T], axis=1)
        y = hd @ ws0 + mean0 @ wn0 + b0
        o2 = np.maximum(y, 0.0)
        if kind == "narrow":
            M = np.zeros((P, N1_TILES * P), np.float32)
            M[:, g * DHALF : (g + 1) * DHALF] = \
                m1n[:, narrow_i * DHALF : (narrow_i + 1) * DHALF]
            narrow_i += 1
        else:
            M = m1full[:, full_i * N1_TILES * P : (full_i + 1) * N1_TILES * P]
            full_i += 1
        mean1T += o2.T @ M
        if t < N1_TILES:
            h1[t * P : (t + 1) * P] = o2

    # layer 1
    m2 = pc["m2"].astype(np.float32).reshape(P, N1_TILES, SEEDS_PER_CORE)
    mean2T = np.zeros((FEAT, SEEDS_PER_CORE), np.float32)
    h2 = np.zeros((N1_TILES * P, FEAT), np.float32)
    for u in range(N1_TILES):
        hd = h1[u * P : (u + 1) * P]
        mean = mean1T[:, u * P : (u + 1) * P].T
        y = hd @ ws1 + mean @ wn1 + b1
        o2 = np.maximum(y, 0.0)
        h2[u * P : (u + 1) * P] = o2
        mean2T += o2.T @ m2[:, u, :]
    # layer 2
    hd = h2[0:SEEDS_PER_CORE]
    y = hd @ ws2 + mean2T.T @ wn2 + b2
    if return_debug:
        return y, dict(h1=h1, mean1T=mean1T, h2=h2, mean2T=mean2T)
    return y


# ---------------------------------------------------------------------------
# Device kernel
# ---------------------------------------------------------------------------


def run_device(meta, trace=False, debug=False):
    import concourse.bacc as bacc
    import concourse.tile as tile
    import concourse.mybir as mybir
    from concourse.bass_utils import run_bass_kernel_spmd

    f32 = mybir.dt.float32
    b16 = mybir.dt.bfloat16
    AF = mybir.ActivationFunctionType
    ALU = mybir.AluOpType

    T0, K, Kmax = meta["T0"], meta["K"], meta["Kmax"]
    sp_off, goff = meta["sp_off"], meta["goff"]
    tile_order, tile_kind = meta["tile_order"], meta["tile_kind"]
    first_t, last_t = meta["first_t"], meta["last_t"]
    n_full = meta["n_full_tiles"]
    n_narrow = meta["n_narrow"]
    ndve, ndma, sp0_off = meta["ndve"], meta["ndma"], meta["sp0_off"]

    nc = bacc.Bacc("TRN2", target_bir_lowering=False, debug=False,
                   num_devices=NCORES)

    band_d = nc.dram_tensor("band", [P, meta["n_band2_cols"]], b16,
                            kind="ExternalInput")
    mdst_d = nc.dram_tensor("mdst", [P, meta["n_sp_cols"]], f32,
                            kind="ExternalInput")
    mval_d = nc.dram_tensor("mval", [P, meta["n_sp_cols"]], f32,
                            kind="ExternalInput")
    m2_d = nc.dram_tensor("m2", [P, N1_TILES * SEEDS_PER_CORE], b16,
                          kind="ExternalInput")
    ident_d = nc.dram_tensor("ident", [P, P], b16, kind="ExternalInput")
    colidx_d = nc.dram_tensor("colidx", [P, P], b16, kind="ExternalInput")
    ones_d = nc.dram_tensor("ones", [1, P], b16, kind="ExternalInput")
    out_d = nc.dram_tensor("out", [SEEDS_PER_CORE, OUTW[2]], f32,
                           kind="ExternalOutput")
    if debug:
        dbg_h1 = nc.dram_tensor("dbg_h1", [N1_TILES * P, FEAT], b16,
                                kind="ExternalOutput")
        dbg_m1 = nc.dram_tensor("dbg_m1", [P, 4 * DHALF], b16,
                                kind="ExternalOutput")
        dbg_h2 = nc.dram_tensor("dbg_h2", [N1_TILES * P, FEAT], b16,
                                kind="ExternalOutput")
        dbg_m2 = nc.dram_tensor("dbg_m2", [P, 2 * SEEDS_PER_CORE], b16,
                                kind="ExternalOutput")
    w_d = []
    for l in range(3):
        w_d.append(
            (
                nc.dram_tensor(f"ws{l}", [FEAT, OUTW[l]], b16,
                               kind="ExternalInput"),
                nc.dram_tensor(f"wn{l}", [FEAT, OUTW[l]], b16,
                               kind="ExternalInput"),
                nc.dram_tensor(f"bias{l}", [1, OUTW[l]], b16,
                               kind="ExternalInput"),
            )
        )

    with tile.TileContext(nc) as tc:
        with (
            tc.tile_pool(name="const", bufs=1) as cpool,
            tc.tile_pool(name="band", bufs=3) as bpool,
            tc.tile_pool(name="mask", bufs=2) as kpool,
            tc.tile_pool(name="o2p", bufs=3) as opool,
            tc.tile_pool(name="aux", bufs=2) as apool,
            tc.tile_pool(name="ps", bufs=1, space="PSUM") as pa,
        ):
            # ---- constants ----
            ident_t = cpool.tile([P, P], b16, tag="ident")
            nc.sync.dma_start(out=ident_t[:], in_=ident_d[:])
            colidx_t = cpool.tile([P, P], b16, tag="colidx")
            nc.sync.dma_start(out=colidx_t[:], in_=colidx_d[:])
            ones_t = cpool.tile([1, P], b16, tag="ones")
            nc.sync.dma_start(out=ones_t[:], in_=ones_d[:])
            mdst_t = cpool.tile([P, meta["n_sp_cols"]], f32, tag="mdst")
            nc.sync.dma_start(out=mdst_t[:], in_=mdst_d[:])
            mval_t = cpool.tile([P, meta["n_sp_cols"]], f32, tag="mval")
            nc.sync.dma_start(out=mval_t[:], in_=mval_d[:])
            m2_t = cpool.tile([P, N1_TILES * SEEDS_PER_CORE], b16, tag="m2")
            nc.sync.dma_start(out=m2_t[:], in_=m2_d[:])
            ws_ts, wn_ts, bias_ts = [], [], []
            for l in range(3):
                outw = OUTW[l]
                wst, wnt = [], []
                for k in range(2):
                    w = cpool.tile([P, outw], b16, tag=f"ws{l}_{k}")
                    nc.sync.dma_start(out=w[:],
                                      in_=w_d[l][0][k * P : (k + 1) * P, :])
                    wst.append(w)
                    w = cpool.tile([P, outw], b16, tag=f"wn{l}_{k}")
                    nc.sync.dma_start(out=w[:],
                                      in_=w_d[l][1][k * P : (k + 1) * P, :])
                    wnt.append(w)
                ws_ts.append(wst)
                wn_ts.append(wnt)
                bias_t = cpool.tile([1, outw], b16, tag=f"bias{l}")
                nc.sync.dma_start(out=bias_t[:], in_=w_d[l][2][:])
                bias_ts.append(bias_t)

            # mean1 accumulators: 4 psum banks [f-half][d-half]
            mean1 = [
                [pa.tile([P, DHALF], f32, tag=f"mean1_{f}_{g}",
                         name=f"mean1_{f}_{g}")
                 for g in range(2)]
                for f in range(2)
            ]
            h1head = [cpool.tile([P, FEAT], b16, tag=f"h1head{u}",
                                 name=f"h1head{u}")
                      for u in range(N1_TILES)]
            h2head = [cpool.tile([P, FEAT], b16, tag=f"h2head{u}",
                                 name=f"h2head{u}")
                      for u in range(N1_TILES)]

            max_cols = max(meta["t_cols"].values())

            def stage_a(t):
                """Band+mask DMA (one fused stripe) + mask gen + agg MMs."""
                Kt = K[t]
                bo = int(meta["boff"][t])
                ncols = int(meta["t_cols"][t])
                bt = bpool.tile([P, max_cols], b16, tag="band")
                nc.gpsimd.dma_start(
                    out=bt[:, :ncols],
                    in_=band_d[:, bo : bo + ncols],
                )
                mk = kpool.tile([P, Kmax * P], b16, tag="mk")
                so = int(sp_off[t])
                for k in range(ndve[t]):
                    nc.vector.tensor_scalar(
                        out=mk[:, k * P : (k + 1) * P],
                        in0=colidx_t[:],
                        scalar1=mdst_t[:, so + k : so + k + 1],
                        scalar2=mval_t[:, so + k : so + k + 1],
                        op0=ALU.is_equal,
                        op1=ALU.mult,
                    )
                mo = (1 + Kt) * FEAT
                mean0 = pa.tile([P, FEAT], f32, tag="mean0", bufs=2)
                for k in range(Kt):
                    lhs = (mk[:, k * P : (k + 1) * P] if k < ndve[t] else
                           bt[:, mo + (k - ndve[t]) * P :
                              mo + (k - ndve[t] + 1) * P])
                    nc.tensor.matmul(
                        mean0[:],
                        lhsT=lhs,
                        rhs=bt[:, (1 + k) * FEAT : (2 + k) * FEAT],
                        start=(k == 0),
                        stop=(k == Kt - 1),
                    )
                return bt, mean0

            def stage_t(t, bt, mean0):
                """Transpose mean0 [d, f] -> meanT [f, d] for tile t."""
                mcopy = apool.tile([P, FEAT], b16, tag="mcopy")
                nc.scalar.activation(out=mcopy[:], in_=mean0[:], func=AF.Copy)
                pt = pa.tile([P, FEAT], b16, tag="pt")
                nc.tensor.transpose(out=pt[:, 0:P], in_=mcopy[:, 0:P],
                                    identity=ident_t[:])
                nc.tensor.transpose(out=pt[:, P:FEAT], in_=mcopy[:, P:FEAT],
                                    identity=ident_t[:])
                mT = apool.tile([P, FEAT], b16, tag="mT")
                nc.scalar.activation(out=mT[:], in_=pt[:], func=AF.Copy)
                return mT

            def stage_c(t, bt, mT):
                """Tail + relu + layer-1 fanout for tile t."""
                kind, g = tile_kind[t]
                # tail
                y = pa.tile([P, OUTW[0]], f32, tag="y")
                nc.tensor.matmul(y[:], lhsT=bt[:, 0:P], rhs=ws_ts[0][0][:],
                                 start=True, stop=False)
                nc.tensor.matmul(y[:], lhsT=bt[:, P:FEAT], rhs=ws_ts[0][1][:],
                                 start=False, stop=False)
                nc.tensor.matmul(y[:], lhsT=mT[:, 0:P], rhs=wn_ts[0][0][:],
                                 start=False, stop=False)
                nc.tensor.matmul(y[:], lhsT=mT[:, P:FEAT], rhs=wn_ts[0][1][:],
                                 start=False, stop=False)
                nc.tensor.matmul(y[:], lhsT=ones_t[0:1, :],
                                 rhs=bias_ts[0][0:1, :],
                                 start=False, stop=True)
                if t < N1_TILES:
                    o2 = h1head[t]
                else:
                    o2 = opool.tile([P, FEAT], b16, tag="o2")
                nc.scalar.activation(out=o2[:], in_=y[:], func=AF.Relu)
                # layer-1 fanout (mask rides the fused band stripe)
                fo = (1 + K[t]) * FEAT + ndma[t] * P
                if kind == "narrow":
                    st = first_t[g] == t
                    sp = last_t[g] == t
                    for f in range(2):
                        nc.tensor.matmul(
                            mean1[f][g][:],
                            lhsT=o2[:, f * P : (f + 1) * P],
                            rhs=bt[:, fo : fo + DHALF],
                            start=st, stop=sp,
                        )
                else:
                    for g2 in range(2):
                        st = first_t[g2] == t
                        sp = last_t[g2] == t
                        for f in range(2):
                            nc.tensor.matmul(
                                mean1[f][g2][:],
                                lhsT=o2[:, f * P : (f + 1) * P],
                                rhs=bt[:, fo + g2 * DHALF :
                                       fo + (g2 + 1) * DHALF],
                                start=st, stop=sp,
                            )

            # ===== layer 0 (3-stage software pipeline: A, T, C) =====
            pipe = []
            for t in tile_order:
                bt, mean0 = stage_a(t)
                pipe.append([t, bt, mean0, None])
                if len(pipe) >= 2:
                    e = pipe[-2]
                    e[3] = stage_t(e[0], e[1], e[2])
                if len(pipe) >= 3:
                    e = pipe.pop(0)
                    stage_c(e[0], e[1], e[3])
            for e in pipe:
                if e[3] is None:
                    e[3] = stage_t(e[0], e[1], e[2])
                stage_c(e[0], e[1], e[3])

            # ================= layer 1 =================
            meanT1 = [
                [cpool.tile([P, DHALF], b16, tag=f"meanT1_{f}_{g}",
                            name=f"meanT1_{f}_{g}")
                 for g in range(2)]
                for f in range(2)
            ]
            for f in range(2):
                for g in range(2):
                    nc.scalar.activation(out=meanT1[f][g][:],
                                         in_=mean1[f][g][:], func=AF.Copy)
            if debug:
                for u in range(N1_TILES):
                    nc.sync.dma_start(out=dbg_h1[u * P : (u + 1) * P, :],
                                      in_=h1head[u][:])
                for f in range(2):
                    for g in range(2):
                        nc.sync.dma_start(
                            out=dbg_m1[:, (2 * f + g) * DHALF :
                                       (2 * f + g + 1) * DHALF],
                            in_=meanT1[f][g][:])

            mean2 = pa.tile([P, FEAT], f32, tag="mean0", bufs=2)
            for u in range(N1_TILES):
                # transpose hd1 tile u
                pt = pa.tile([P, FEAT], b16, tag="pt")
                nc.tensor.transpose(out=pt[:, 0:P], in_=h1head[u][:, 0:P],
                                    identity=ident_t[:])
                nc.tensor.transpose(out=pt[:, P:FEAT],
                                    in_=h1head[u][:, P:FEAT],
                                    identity=ident_t[:])
                hdT = apool.tile([P, FEAT], b16, tag="mT")
                nc.scalar.activation(out=hdT[:], in_=pt[:], func=AF.Copy)
                y = pa.tile([P, OUTW[1]], f32, tag="y")
                g, j = u // 3, u % 3
                nc.tensor.matmul(y[:], lhsT=hdT[:, 0:P], rhs=ws_ts[1][0][:],
                                 start=True, stop=False)
                nc.tensor.matmul(y[:], lhsT=hdT[:, P:FEAT],
                                 rhs=ws_ts[1][1][:], start=False, stop=False)
                nc.tensor.matmul(y[:],
                                 lhsT=meanT1[0][g][:, j * P : (j + 1) * P],
                                 rhs=wn_ts[1][0][:], start=False, stop=False)
                nc.tensor.matmul(y[:],
                                 lhsT=meanT1[1][g][:, j * P : (j + 1) * P],
                                 rhs=wn_ts[1][1][:], start=False, stop=False)
                nc.tensor.matmul(y[:], lhsT=ones_t[0:1, :],
                                 rhs=bias_ts[1][0:1, :],
                                 start=False, stop=True)
                nc.scalar.activation(out=h2head[u][:], in_=y[:], func=AF.Relu)
                # layer-2 fanout: mean2 cols [0:64]=f0, [128:192]=f1
                # NOTE: start=True clears has_written for the WHOLE bank, so
                # only the very first matmul into this bank may carry it.
                for f in range(2):
                    nc.tensor.matmul(
                        mean2[:, f * P : f * P + SEEDS_PER_CORE],
                        lhsT=h2head[u][:, f * P : (f + 1) * P],
                        rhs=m2_t[:, u * SEEDS_PER_CORE : (u + 1) * SEEDS_PER_CORE],
                        start=(u == 0 and f == 0),
                        stop=(u == N1_TILES - 1),
                    )

            # ================= layer 2 =================
            mT2 = apool.tile([P, 2 * SEEDS_PER_CORE], b16, tag="mT2")
            nc.scalar.activation(out=mT2[:, 0:SEEDS_PER_CORE],
                                 in_=mean2[:, 0:SEEDS_PER_CORE], func=AF.Copy)
            nc.scalar.activation(out=mT2[:, SEEDS_PER_CORE:],
                                 in_=mean2[:, P : P + SEEDS_PER_CORE],
                                 func=AF.Copy)
            pt = pa.tile([P, FEAT], b16, tag="pt")
            nc.tensor.transpose(out=pt[:, 0:P], in_=h2head[0][:, 0:P],
                                identity=ident_t[:])
            nc.tensor.transpose(out=pt[:, P:FEAT], in_=h2head[0][:, P:FEAT],
                                identity=ident_t[:])
            hd2 = apool.tile([P, FEAT], b16, tag="mcopy")
            nc.scalar.activation(out=hd2[:], in_=pt[:], func=AF.Copy)
            y2 = pa.tile([P, OUTW[0]], f32, tag="y")
            S = SEEDS_PER_CORE
            nc.tensor.matmul(y2[0:S, 0:OUTW[2]], lhsT=hd2[:, 0:S],
                             rhs=ws_ts[2][0][:], start=True, stop=False)
            nc.tensor.matmul(y2[0:S, 0:OUTW[2]], lhsT=hd2[:, P : P + S],
                             rhs=ws_ts[2][1][:], start=False, stop=False)
            nc.tensor.matmul(y2[0:S, 0:OUTW[2]], lhsT=mT2[:, 0:S],
                             rhs=wn_ts[2][0][:], start=False, stop=False)
            nc.tensor.matmul(y2[0:S, 0:OUTW[2]], lhsT=mT2[:, S : 2 * S],
                             rhs=wn_ts[2][1][:], start=False, stop=False)
            nc.tensor.matmul(y2[0:S, 0:OUTW[2]], lhsT=ones_t[0:1, 0:S],
                             rhs=bias_ts[2][0:1, :], start=False, stop=True)
            o_f32 = apool.tile([P, OUTW[2]], f32, tag="ofin")
            nc.vector.tensor_copy(out=o_f32[0:S, :], in_=y2[0:S, 0:OUTW[2]])
            nc.sync.dma_start(out=out_d[:], in_=o_f32[0:S, :])
            if debug:
                for u in range(N1_TILES):
                    nc.sync.dma_start(out=dbg_h2[u * P : (u + 1) * P, :],
                                      in_=h2head[u][:])
                nc.sync.dma_start(out=dbg_m2[:], in_=mT2[:])

    nc.compile()

    bf16 = _bf16()
    eye16 = np.eye(P, dtype=bf16)
    colidx16 = np.broadcast_to(
        np.arange(P, dtype=np.float32), (P, P)
    ).astype(bf16)
    in_maps = []
    for c in range(NCORES):
        pc = meta["per_core"][c]
        m = dict(
            band=pc["band2"],
            mdst=pc["meta_dst"],
            mval=pc["meta_val"],
            m2=pc["m2"],
            ident=eye16,
            colidx=np.ascontiguousarray(colidx16),
            ones=np.ones((1, P), dtype=bf16),
        )
        for l in range(3):
            ws, wn, b = meta["weights"][l]
            m[f"ws{l}"] = np.ascontiguousarray(ws.astype(bf16))
            m[f"wn{l}"] = np.ascontiguousarray(wn.astype(bf16))
            m[f"bias{l}"] = np.ascontiguousarray(b[None, :].astype(bf16))
        in_maps.append(m)

    res = run_bass_kernel_spmd(
        nc, in_maps, core_ids=list(range(NCORES)), trace=trace
    )
    if debug:
        return [res.results[c] for c in range(NCORES)], res
    return [res.results[c]["out"] for c in range(NCORES)], res


def assemble(meta, outs):
    full = np.zeros((NUM_DST[2], OUTW[2]), np.float32)
    for c in range(NCORES):
        full[meta["blocks"][c]["seeds"]] = outs[c]
    return full


def kernel(**inputs) -> np.ndarray:
    meta = build_host(inputs)
    outs, _ = run_device(meta)
    return assemble(meta, outs)


# revision 29
# speedup vs baseline: 1.3037x; 1.3037x over previous
"""DistSAGE 3-layer GraphSAGE forward on 8 TRN2 NeuronCores (Bass/Tile).

Strategy (graph/data parallel, per the DistSAGE recipe):
  - Partition the 512 seed nodes across 8 cores (64 each, LPT-balanced by
    an additive 2-hop cost estimate); build per-core dependency-driven
    blocks on the host. No inter-core communication; weights replicated.
  - Layer 0 streams per-dst-tile dense bands (dst rows stored transposed
    + per-edge source rows, bf16, pre-interleaved for line-rate DMA).
    The S' aggregation masks are GENERATED ON-CHIP by the vector engine
    (one-hot: (colidx == dstcol[p]) * inv_deg[p]) from tiny per-chunk
    metadata, then used as the STATIONARY matmul operand with the 256-wide
    message groups streaming: mean0[d, f] += mask_k.T @ msgs_k.
  - No DRAM round-trip between layers: as each layer-0 output tile is
    ReLU'd in SBUF, small fanout matmuls scatter-accumulate its
    contribution into layer-1 mean accumulators held in PSUM
    (meanT1[f, d1] += o2_chunk.T @ M1_tile). Extra (non-l1) nodes are
    grouped by primary layer-1 dst half so most fanouts are narrow; the
    l1_out-region tiles (and multi-half nodes) use full-width host-baked
    masks. Layers 1/2 then finish with a ~10us tail (h1/h2 stay in SBUF).
"""

import heapq

import numpy as np

P = 128
NCORES = 8
NUM_DST = (61952, 5632, 512)
FEAT = 256
OUTW = (256, 256, 19)
SEEDS_PER_CORE = NUM_DST[2] // NCORES  # 64
N1_TILES = 6  # layer-1 dst tiles (n1 <= 768 on every core; asserted)
DHALF = N1_TILES * P // 2  # 384
PAD_DST = 200.0  # one-hot "never matches" sentinel column index


def _bf16():
    import ml_dtypes

    return ml_dtypes.bfloat16


# ---------------------------------------------------------------------------
# Host-side block construction
# ---------------------------------------------------------------------------


def _balance(ids, deg, n_buckets):
    """LPT bin-packing: reorder ids so consecutive 128-groups have ~equal
    total degree (only full 128-groups are balanced)."""
    if n_buckets <= 1 or len(ids) < n_buckets * P:
        return ids
    order = np.argsort(-deg[ids], kind="stable")
    heap = [(0.0, b, 0) for b in range(n_buckets)]
    heapq.heapify(heap)
    buckets = [[] for _ in range(n_buckets)]
    for i in order:
        load, b, cnt = heapq.heappop(heap)
        buckets[b].append(ids[i])
        cnt += 1
        if cnt < P:
            heapq.heappush(heap, (load + deg[ids[i]], b, cnt))
    return np.concatenate([np.asarray(b, dtype=ids.dtype) for b in buckets])


def _seed_partition(esrc0, edst0, esrc1, edst1, esrc2, edst2, deg0, deg1):
    """LPT-balance seeds across cores by an additive 2-hop cost estimate."""
    h = np.zeros(NUM_DST[1], np.float64)
    np.add.at(h, edst1, deg0[esrc1].astype(np.float64))
    cost = np.zeros(NUM_DST[2], np.float64)
    np.add.at(cost, edst2, h[esrc2] + deg1[esrc2].astype(np.float64))
    order = np.argsort(-cost, kind="stable")
    heap = [(0.0, cc, 0) for cc in range(NCORES)]
    heapq.heapify(heap)
    groups = [[] for _ in range(NCORES)]
    for s in order:
        load, cc, cnt = heapq.heappop(heap)
        groups[cc].append(s)
        cnt += 1
        if cnt < SEEDS_PER_CORE:
            heapq.heappush(heap, (load + cost[s], cc, cnt))
    return [np.array(g, dtype=np.int64) for g in groups]


def _block_for_core(seeds, esrc0, edst0, esrc1, edst1, esrc2, edst2,
                    deg0, deg1):
    """Raw per-core block: l1_out, l0 extras split by primary l1 d-half,
    and the edge lists (in global node ids / l1 positions)."""
    pos2 = np.full(NUM_DST[2], -1, np.int32)
    pos2[seeds] = np.arange(SEEDS_PER_CORE, dtype=np.int32)
    sel2 = pos2[edst2] >= 0
    es2, ed2g = esrc2[sel2], edst2[sel2]
    l1_extra = np.setdiff1d(np.unique(es2), seeds)
    l1_out = np.concatenate([seeds, l1_extra])
    n1 = len(l1_out)
    assert n1 <= N1_TILES * P

    pos1 = np.full(NUM_DST[1], -1, np.int32)
    pos1[l1_out] = np.arange(n1, dtype=np.int32)
    sel1 = pos1[edst1] >= 0
    es1, ed1g = esrc1[sel1], edst1[sel1]
    ed1 = pos1[ed1g].astype(np.int64)  # l1 positions [0, n1)
    inv1 = (1.0 / np.maximum(deg1[ed1g], 1.0)).astype(np.float32)

    l0_extra = np.setdiff1d(np.unique(es1), l1_out)

    # primary d-half per extra node: halves touched by its l1 edges
    emask = np.zeros((NUM_DST[0], 2), bool)
    np.logical_or.at(emask, (es1, np.minimum(ed1 // DHALF, 1)), True)
    m0 = emask[l0_extra, 0]
    m1 = emask[l0_extra, 1]
    g_both = l0_extra[m0 & m1]
    g0 = l0_extra[m0 & ~m1]
    g1 = l0_extra[~m0 & m1]

    ed2 = pos2[ed2g].astype(np.int64)
    inv2 = (1.0 / np.maximum(deg2_of(ed2g, edst2), 1.0)).astype(np.float32)
    es2l = pos1[es2].astype(np.int64)

    return dict(
        seeds=seeds, l1_out=l1_out, n1=n1,
        g0=g0, g1=g1, g_both=g_both,
        es1=es1, ed1=ed1, inv1=inv1,
        es2l=es2l, ed2=ed2, inv2=inv2,
    )


_DEG2 = None


def deg2_of(ids, edst2):
    global _DEG2
    if _DEG2 is None:
        _DEG2 = np.bincount(edst2, minlength=NUM_DST[2]).astype(np.float32)
    return _DEG2[ids]


def build_host(inputs):
    global _DEG2
    _DEG2 = None
    esrc0 = np.asarray(inputs["esrc0"]).astype(np.int64)
    edst0 = np.asarray(inputs["edst0"]).astype(np.int64)
    esrc1 = np.asarray(inputs["esrc1"]).astype(np.int64)
    edst1 = np.asarray(inputs["edst1"]).astype(np.int64)
    esrc2 = np.asarray(inputs["esrc2"]).astype(np.int64)
    edst2 = np.asarray(inputs["edst2"]).astype(np.int64)
    x = np.asarray(inputs["x"], dtype=np.float32)

    deg0 = np.bincount(edst0, minlength=NUM_DST[0]).astype(np.float32)
    deg1 = np.bincount(edst1, minlength=NUM_DST[1]).astype(np.float32)

    seed_groups = _seed_partition(esrc0, edst0, esrc1, edst1, esrc2, edst2,
                                  deg0, deg1)
    blocks = [
        _block_for_core(seed_groups[c], esrc0, edst0, esrc1, edst1, esrc2,
                        edst2, deg0, deg1)
        for c in range(NCORES)
    ]

    # ---- uniform group sizes (padded to max over cores, then to 128) ----
    def padlen(key):
        m = max(len(b[key]) for b in blocks)
        return -(-max(m, 1) // P) * P if m > 0 else 0

    L0, L1, LB = padlen("g0"), padlen("g1"), padlen("g_both")
    NL1 = N1_TILES * P  # 768 rows for the l1_out region

    # row layout in l0_out: [l1_out(768) | g0(L0) | g1(L1) | both(LB)]
    # per-core l0 node list (global ids), padded with dummy = l1_out[0]
    l0_rows = []
    def balanced(ids, deg):
        nfull = (len(ids) // P) * P
        if nfull < P:
            return ids
        return np.concatenate([_balance(ids[:nfull], deg, nfull // P),
                               ids[nfull:]])

    for b in blocks:
        dummy = b["l1_out"][0]
        g0b = balanced(b["g0"], deg0) if L0 else np.zeros(0, np.int64)
        g0v = np.full(L0, dummy, np.int64)
        g0v[: len(g0b)] = g0b
        g1b = balanced(b["g1"], deg0) if L1 else np.zeros(0, np.int64)
        g1v = np.full(L1, dummy, np.int64)
        g1v[: len(g1b)] = g1b
        gbv = np.full(LB, dummy, np.int64)
        gbv[: len(b["g_both"])] = b["g_both"]
        l1v = np.full(NL1, dummy, np.int64)
        l1v[: b["n1"]] = b["l1_out"]
        l0_rows.append(np.concatenate([l1v, g0v, g1v, gbv]))

    n0_pad = NL1 + L0 + L1 + LB
    T0 = n0_pad // P

    # tile processing order: narrow g0 tiles, narrow g1 tiles, full tiles
    # (l1_out region first 6 + both-group tiles last)
    t_g0 = list(range(N1_TILES, N1_TILES + L0 // P))
    t_g1 = list(range(N1_TILES + L0 // P, N1_TILES + (L0 + L1) // P))
    t_l1 = list(range(N1_TILES))
    t_b = list(range(N1_TILES + (L0 + L1) // P, T0))
    tile_order = t_g0 + t_g1 + t_l1 + t_b
    tile_kind = {}  # tile -> ("narrow", g) | ("full", None)
    for t in t_g0:
        tile_kind[t] = ("narrow", 0)
    for t in t_g1:
        tile_kind[t] = ("narrow", 1)
    for t in t_l1 + t_b:
        tile_kind[t] = ("full", None)

    # ---- layer-0 per-tile edge slots (per-edge; dedup not needed) ----
    # per core: positions of l0 rows for mapping layer-0 edges (first
    # occurrence wins; duplicate rows are dummy pads and receive no edges)
    pos0s = []
    for c in range(NCORES):
        rows = l0_rows[c]
        uniq, first_idx = np.unique(rows, return_index=True)
        pos0 = np.full(NUM_DST[0], -1, np.int64)
        pos0[uniq] = first_idx
        pos0s.append(pos0)

    # per-core, per-tile layer-0 edges: (x_row, dstcol, inv)
    tile_edges = [[None] * T0 for _ in range(NCORES)]
    for c, b in enumerate(blocks):
        pos0 = pos0s[c]
        sel0 = pos0[edst0] >= 0
        es0, ed0g = esrc0[sel0], edst0[sel0]
        ed0 = pos0[ed0g]
        inv0 = (1.0 / np.maximum(deg0[ed0g], 1.0)).astype(np.float32)
        tt = ed0 // P
        order = np.argsort(tt, kind="stable")
        es0, ed0, inv0, tt = es0[order], ed0[order], inv0[order], tt[order]
        starts = np.searchsorted(tt, np.arange(T0))
        ends = np.searchsorted(tt, np.arange(T0) + 1)
        for t in range(T0):
            s, e = starts[t], ends[t]
            tile_edges[c][t] = (es0[s:e], (ed0[s:e] - t * P), inv0[s:e])

    K = [
        max(1, max(-(-len(tile_edges[c][t][0]) // P) for c in range(NCORES)))
        for t in range(T0)
    ]
    Kmax = max(K)
    sp_off = np.concatenate([[0], np.cumsum(K)]).astype(np.int64)
    n_sp_cols = int(sp_off[-1])
    goff = np.concatenate([[0], np.cumsum([1 + k for k in K])]).astype(np.int64)
    n_groups = int(goff[-1])

    # ---- layer-1 fanout plans ----
    # narrow tiles: 3 base passes (+ uniform extra passes) of one-hot DVE gen
    # full tiles: host-baked dense [128, 768] masks
    # per core per tile: list of (row, dst_l1pos, val)
    l1fan = [[None] * T0 for _ in range(NCORES)]
    for c, b in enumerate(blocks):
        pos0 = pos0s[c]
        src_r = pos0[b["es1"]]
        assert (src_r >= 0).all()
        tt = src_r // P
        order = np.argsort(tt, kind="stable")
        sr, dd, vv, tt = (src_r[order], b["ed1"][order], b["inv1"][order],
                          tt[order])
        starts = np.searchsorted(tt, np.arange(T0))
        ends = np.searchsorted(tt, np.arange(T0) + 1)
        for t in range(T0):
            s, e = starts[t], ends[t]
            l1fan[c][t] = (sr[s:e] - t * P, dd[s:e], vv[s:e])

    # layer-0 agg chunk roles: first ndve chunks DVE-generated, rest DMA'd
    DVE_FRAC = 0.70
    ndve = [max(1, min(K[t], int(round(K[t] * DVE_FRAC)))) for t in range(T0)]
    ndma = [K[t] - ndve[t] for t in range(T0)]
    sp0_off = np.concatenate([[0], np.cumsum(ndma)]).astype(np.int64)
    n_sp0_cols = int(sp0_off[-1])
    n_narrow = sum(1 for t in tile_order if tile_kind[t][0] == "narrow")

    # fused band2 column layout (per processing-order tile)
    t_cols = {}
    for t in tile_order:
        kind, g = tile_kind[t]
        fw = DHALF if kind == "narrow" else N1_TILES * P
        t_cols[t] = FEAT * (1 + K[t]) + ndma[t] * P + fw
    boff = {}
    acc = 0
    for t in tile_order:
        boff[t] = acc
        acc += t_cols[t]
    n_band2_cols = acc

    bf16 = _bf16()
    x16 = x.astype(bf16)

    per_core = []
    for c in range(NCORES):
        b = blocks[c]
        band = np.zeros((P, n_groups, FEAT), bf16)
        meta_dst = np.full((P, n_sp_cols), PAD_DST, np.float32)
        meta_val = np.zeros((P, n_sp_cols), np.float32)
        for t in range(T0):
            g0c = int(goff[t])
            rows = l0_rows[c][t * P : (t + 1) * P]
            blk = np.ascontiguousarray(x16[rows])  # [128, 256]
            band[:, g0c, 0:P] = blk[:, 0:P].T
            band[:, g0c, P:FEAT] = blk[:, P:FEAT].T
            es, dc, iv = tile_edges[c][t]
            ne = len(es)
            for k in range(K[t]):
                a, e2 = k * P, min((k + 1) * P, ne)
                if a >= e2:
                    break
                band[: e2 - a, g0c + 1 + k, :] = x16[es[a:e2]]
                meta_dst[: e2 - a, int(sp_off[t]) + k] = dc[a:e2]
                meta_val[: e2 - a, int(sp_off[t]) + k] = iv[a:e2]

        # baked dense masks for the DMA-assigned layer-0 chunks
        sp0 = np.zeros((P, n_sp0_cols, P), np.float32)
        for t in range(T0):
            es, dc, iv = tile_edges[c][t]
            ne = len(es)
            for j in range(ndma[t]):
                k = ndve[t] + j
                a, e2 = k * P, min((k + 1) * P, ne)
                if a >= e2:
                    continue
                sp0[np.arange(e2 - a), int(sp0_off[t]) + j, dc[a:e2]] = iv[a:e2]

        # layer-1 fanout masks: narrow [128, 384] + full [128, 768], dense
        m1n, m1full = [], []
        for t in tile_order:
            kind, g = tile_kind[t]
            rows, dd, vv = l1fan[c][t]
            if kind == "narrow":
                W = np.zeros((P, DHALF), np.float32)
                np.add.at(W, (rows, dd - g * DHALF), vv)
                m1n.append(W.astype(bf16))
            else:
                W = np.zeros((P, N1_TILES * P), np.float32)
                np.add.at(W, (rows, dd), vv)
                m1full.append(W.astype(bf16))
        m1n = (np.concatenate(m1n, axis=1) if m1n
               else np.zeros((P, 0), bf16))
        m1full = (np.concatenate(m1full, axis=1) if m1full
                  else np.zeros((P, 0), bf16))

        # layer-2 fanout masks [6][128, 64]
        m2 = np.zeros((P, N1_TILES, SEEDS_PER_CORE), np.float32)
        u = b["es2l"] // P
        r2 = b["es2l"] % P
        np.add.at(m2, (r2, u, b["ed2"]), b["inv2"])

        # fused per-tile stripe: [hdT | msgs | dma-masks | fanout mask]
        sp016 = sp0.reshape(P, n_sp0_cols * P).astype(bf16)
        band2 = np.zeros((P, n_band2_cols), bf16)
        ni2 = fi2 = 0
        for t in tile_order:
            kind, g = tile_kind[t]
            bo = int(boff[t])
            nb = FEAT * (1 + K[t])
            band2[:, bo : bo + nb] = band.reshape(P, n_groups * FEAT)[
                :, int(goff[t]) * FEAT : int(goff[t]) * FEAT + nb]
            mo = bo + nb
            if ndma[t]:
                band2[:, mo : mo + ndma[t] * P] = sp016[
                    :, int(sp0_off[t]) * P : (int(sp0_off[t]) + ndma[t]) * P]
            fo = mo + ndma[t] * P
            if kind == "narrow":
                band2[:, fo : fo + DHALF] = m1n[
                    :, ni2 * DHALF : (ni2 + 1) * DHALF]
                ni2 += 1
            else:
                band2[:, fo : fo + N1_TILES * P] = m1full[
                    :, fi2 * N1_TILES * P : (fi2 + 1) * N1_TILES * P]
                fi2 += 1
        per_core.append(dict(
            band=np.ascontiguousarray(band.reshape(P, n_groups * FEAT)),
            band2=np.ascontiguousarray(band2),
            meta_dst=meta_dst,
            meta_val=meta_val,
            sp0=np.ascontiguousarray(sp016),
            m1n=np.ascontiguousarray(m1n),
            m1full=np.ascontiguousarray(m1full),
            m2=np.ascontiguousarray(
                m2.reshape(P, N1_TILES * SEEDS_PER_CORE).astype(bf16)
            ),
        ))

    n_full_tiles = sum(1 for t in tile_order if tile_kind[t][0] == "full")

    # start/stop flags for the mean1 accumulator chunks (keyed by d-half g)
    first_t = {0: None, 1: None}
    last_t = {0: None, 1: None}
    for t in tile_order:
        kind, g = tile_kind[t]
        gs = [g] if kind == "narrow" else [0, 1]
        for gg in gs:
            if first_t[gg] is None:
                first_t[gg] = t
            last_t[gg] = t

    return dict(
        blocks=blocks,
        T0=T0, K=K, Kmax=Kmax, sp_off=sp_off, n_sp_cols=n_sp_cols,
        goff=goff, n_groups=n_groups,
        tile_order=tile_order, tile_kind=tile_kind,
        ndve=ndve, ndma=ndma, sp0_off=sp0_off, n_sp0_cols=n_sp0_cols,
        n_narrow=n_narrow, boff=boff, t_cols=t_cols,
        n_band2_cols=n_band2_cols,
        n_full_tiles=n_full_tiles,
        first_t=first_t, last_t=last_t,
        per_core=per_core,
        n0_pad=n0_pad,
        weights=tuple(
            (
                np.asarray(inputs[f"W_self{l}"], np.float32),
                np.asarray(inputs[f"W_neigh{l}"], np.float32),
                np.asarray(inputs[f"b{l}"], np.float32),
            )
            for l in range(3)
        ),
    )


# ---------------------------------------------------------------------------
# Numpy simulation of the device kernel (validation aid; fp32 stand-in)
# ---------------------------------------------------------------------------


def simulate_core(meta, c, return_debug=False):
    pc = meta["per_core"][c]
    T0, K, sp_off, goff = meta["T0"], meta["K"], meta["sp_off"], meta["goff"]
    band = pc["band"].astype(np.float32).reshape(P, meta["n_groups"], FEAT)
    colidx = np.arange(P, dtype=np.float32)

    ws0, wn0, b0 = meta["weights"][0]
    ws1, wn1, b1 = meta["weights"][1]
    ws2, wn2, b2 = meta["weights"][2]

    mean1T = np.zeros((FEAT, N1_TILES * P), np.float32)
    h1 = np.zeros((N1_TILES * P, FEAT), np.float32)
    full_i = 0
    narrow_i = 0
    m1full = pc["m1full"].astype(np.float32)
    m1n = pc["m1n"].astype(np.float32)
    sp0 = pc["sp0"].astype(np.float32).reshape(P, -1, P)
    ndve, ndma, sp0_off = meta["ndve"], meta["ndma"], meta["sp0_off"]
    for t in meta["tile_order"]:
        kind, g = meta["tile_kind"][t]
        g0c = int(goff[t])
        hdT = np.concatenate(
            [band[:, g0c, 0:P], band[:, g0c, P:FEAT]], axis=1
        )  # [128f, 2*128d] halves
        mean0 = np.zeros((P, FEAT), np.float32)
        for k in range(K[t]):
            if k < ndve[t]:
                dst = pc["meta_dst"][:, int(sp_off[t]) + k].astype(np.float32)
                val = pc["meta_val"][:, int(sp_off[t]) + k].astype(np.float32)
                mask = (colidx[None, :] == dst[:, None]) * val[:, None]
            else:
                mask = sp0[:, int(sp0_off[t]) + (k - ndve[t]), :]
            mean0 += mask.T @ band[:, g0c + 1 + k, :]
        hd = np.concatenate([hdT[:, 0:P].T, hdT[:, P : 2 * P].T], axis=1)
        y = hd @ ws0 + mean0 @ wn0 + b0
        o2 = np.maximum(y, 0.0)
        if kind == "narrow":
            M = np.zeros((P, N1_TILES * P), np.float32)
            M[:, g * DHALF : (g + 1) * DHALF] = \
                m1n[:, narrow_i * DHALF : (narrow_i + 1) * DHALF]
            narrow_i += 1
        else:
            M = m1full[:, full_i * N1_TILES * P : (full_i + 1) * N1_TILES * P]
            full_i += 1
        mean1T += o2.T @ M
        if t < N1_TILES:
            h1[t * P : (t + 1) * P] = o2

    # layer 1
    m2 = pc["m2"].astype(np.float32).reshape(P, N1_TILES, SEEDS_PER_CORE)
    mean2T = np.zeros((FEAT, SEEDS_PER_CORE), np.float32)
    h2 = np.zeros((N1_TILES * P, FEAT), np.float32)
    for u in range(N1_TILES):
        hd = h1[u * P : (u + 1) * P]
        mean = mean1T[:, u * P : (u + 1) * P].T
        y = hd @ ws1 + mean @ wn1 + b1
        o2 = np.maximum(y, 0.0)
        h2[u * P : (u + 1) * P] = o2
        mean2T += o2.T @ m2[:, u, :]
    # layer 2
    hd = h2[0:SEEDS_PER_CORE]
    y = hd @ ws2 + mean2T.T @ wn2 + b2
    if return_debug:
        return y, dict(h1=h1, mean1T=mean1T, h2=h2, mean2T=mean2T)
    return y


# ---------------------------------------------------------------------------
# Device kernel
# ---------------------------------------------------------------------------


def run_device(meta, trace=False, debug=False):
    import concourse.bacc as bacc
    import concourse.tile as tile
    import concourse.mybir as mybir
    from concourse.bass_utils import run_bass_kernel_spmd

    f32 = mybir.dt.float32
    b16 = mybir.dt.bfloat16
    AF = mybir.ActivationFunctionType
    ALU = mybir.AluOpType

    T0, K, Kmax = meta["T0"], meta["K"], meta["Kmax"]
    sp_off, goff = meta["sp_off"], meta["goff"]
    tile_order, tile_kind = meta["tile_order"], meta["tile_kind"]
    first_t, last_t = meta["first_t"], meta["last_t"]
    n_full = meta["n_full_tiles"]
    n_narrow = meta["n_narrow"]
    ndve, ndma, sp0_off = meta["ndve"], meta["ndma"], meta["sp0_off"]

    nc = bacc.Bacc("TRN2", target_bir_lowering=False, debug=False,
                   num_devices=NCORES)

    band_d = nc.dram_tensor("band", [P, meta["n_band2_cols"]], b16,
                            kind="ExternalInput")
    mdst_d = nc.dram_tensor("mdst", [P, meta["n_sp_cols"]], f32,
                            kind="ExternalInput")
    mval_d = nc.dram_tensor("mval", [P, meta["n_sp_cols"]], f32,
                            kind="ExternalInput")
    m2_d = nc.dram_tensor("m2", [P, N1_TILES * SEEDS_PER_CORE], b16,
                          kind="ExternalInput")
    ident_d = nc.dram_tensor("ident", [P, P], b16, kind="ExternalInput")
    colidx_d = nc.dram_tensor("colidx", [P, P], b16, kind="ExternalInput")
    ones_d = nc.dram_tensor("ones", [1, P], b16, kind="ExternalInput")
    out_d = nc.dram_tensor("out", [SEEDS_PER_CORE, OUTW[2]], f32,
                           kind="ExternalOutput")
    if debug:
        dbg_h1 = nc.dram_tensor("dbg_h1", [N1_TILES * P, FEAT], b16,
                                kind="ExternalOutput")
        dbg_m1 = nc.dram_tensor("dbg_m1", [P, 4 * DHALF], b16,
                                kind="ExternalOutput")
        dbg_h2 = nc.dram_tensor("dbg_h2", [N1_TILES * P, FEAT], b16,
                                kind="ExternalOutput")
        dbg_m2 = nc.dram_tensor("dbg_m2", [P, 2 * SEEDS_PER_CORE], b16,
                                kind="ExternalOutput")
    w_d = []
    for l in range(3):
        w_d.append(
            (
                nc.dram_tensor(f"ws{l}", [FEAT, OUTW[l]], b16,
                               kind="ExternalInput"),
                nc.dram_tensor(f"wn{l}", [FEAT, OUTW[l]], b16,
                               kind="ExternalInput"),
                nc.dram_tensor(f"bias{l}", [1, OUTW[l]], b16,
                               kind="ExternalInput"),
            )
        )

    with tile.TileContext(nc) as tc:
        with (
            tc.tile_pool(name="const", bufs=1) as cpool,
            tc.tile_pool(name="band", bufs=6) as bpool,
            tc.tile_pool(name="mask", bufs=2) as kpool,
            tc.tile_pool(name="o2p", bufs=3) as opool,
            tc.tile_pool(name="aux", bufs=2) as apool,
            tc.tile_pool(name="ps", bufs=1, space="PSUM") as pa,
        ):
            # ---- constants ----
            ident_t = cpool.tile([P, P], b16, tag="ident")
            nc.sync.dma_start(out=ident_t[:], in_=ident_d[:])
            colidx_t = cpool.tile([P, P], b16, tag="colidx")
            nc.sync.dma_start(out=colidx_t[:], in_=colidx_d[:])
            ones_t = cpool.tile([1, P], b16, tag="ones")
            nc.sync.dma_start(out=ones_t[:], in_=ones_d[:])
            mdst_t = cpool.tile([P, meta["n_sp_cols"]], f32, tag="mdst")
            nc.sync.dma_start(out=mdst_t[:], in_=mdst_d[:])
            mval_t = cpool.tile([P, meta["n_sp_cols"]], f32, tag="mval")
            nc.sync.dma_start(out=mval_t[:], in_=mval_d[:])
            m2_t = cpool.tile([P, N1_TILES * SEEDS_PER_CORE], b16, tag="m2")
            nc.sync.dma_start(out=m2_t[:], in_=m2_d[:])
            ws_ts, wn_ts, bias_ts = [], [], []
            for l in range(3):
                outw = OUTW[l]
                wst, wnt = [], []
                for k in range(2):
                    w = cpool.tile([P, outw], b16, tag=f"ws{l}_{k}")
                    nc.sync.dma_start(out=w[:],
                                      in_=w_d[l][0][k * P : (k + 1) * P, :])
                    wst.append(w)
                    w = cpool.tile([P, outw], b16, tag=f"wn{l}_{k}")
                    nc.sync.dma_start(out=w[:],
                                      in_=w_d[l][1][k * P : (k + 1) * P, :])
                    wnt.append(w)
                ws_ts.append(wst)
                wn_ts.append(wnt)
                bias_t = cpool.tile([1, outw], b16, tag=f"bias{l}")
                nc.sync.dma_start(out=bias_t[:], in_=w_d[l][2][:])
                bias_ts.append(bias_t)

            # mean1 accumulators: 4 psum banks [f-half][d-half]
            mean1 = [
                [pa.tile([P, DHALF], f32, tag=f"mean1_{f}_{g}",
                         name=f"mean1_{f}_{g}")
                 for g in range(2)]
                for f in range(2)
            ]
            h1head = [cpool.tile([P, FEAT], b16, tag=f"h1head{u}",
                                 name=f"h1head{u}")
                      for u in range(N1_TILES)]
            h2head = [cpool.tile([P, FEAT], b16, tag=f"h2head{u}",
                                 name=f"h2head{u}")
                      for u in range(N1_TILES)]

            max_cols = max(meta["t_cols"].values())

            def stage_d(t):
                """Issue the fused band+mask stripe DMA for tile t."""
                bo = int(meta["boff"][t])
                ncols = int(meta["t_cols"][t])
                bt = bpool.tile([P, max_cols], b16, tag="band")
                nc.gpsimd.dma_start(
                    out=bt[:, :ncols],
                    in_=band_d[:, bo : bo + ncols],
                )
                return bt

            def stage_a(t, bt):
                """Mask gen + aggregation matmuls for tile t."""
                Kt = K[t]
                mk = kpool.tile([P, Kmax * P], b16, tag="mk")
                so = int(sp_off[t])
                for k in range(ndve[t]):
                    nc.vector.tensor_scalar(
                        out=mk[:, k * P : (k + 1) * P],
                        in0=colidx_t[:],
                        scalar1=mdst_t[:, so + k : so + k + 1],
                        scalar2=mval_t[:, so + k : so + k + 1],
                        op0=ALU.is_equal,
                        op1=ALU.mult,
                    )
                mo = (1 + Kt) * FEAT
                mean0 = pa.tile([P, FEAT], f32, tag="mean0", bufs=2)
                for k in range(Kt):
                    lhs = (mk[:, k * P : (k + 1) * P] if k < ndve[t] else
                           bt[:, mo + (k - ndve[t]) * P :
                              mo + (k - ndve[t] + 1) * P])
                    nc.tensor.matmul(
                        mean0[:],
                        lhsT=lhs,
                        rhs=bt[:, (1 + k) * FEAT : (2 + k) * FEAT],
                        start=(k == 0),
                        stop=(k == Kt - 1),
                    )
                return mean0

            def stage_t(t, bt, mean0):
                """Transpose mean0 [d, f] -> meanT [f, d] for tile t."""
                mcopy = apool.tile([P, FEAT], b16, tag="mcopy")
                nc.scalar.activation(out=mcopy[:], in_=mean0[:], func=AF.Copy)
                pt = pa.tile([P, FEAT], b16, tag="pt")
                nc.tensor.transpose(out=pt[:, 0:P], in_=mcopy[:, 0:P],
                                    identity=ident_t[:])
                nc.tensor.transpose(out=pt[:, P:FEAT], in_=mcopy[:, P:FEAT],
                                    identity=ident_t[:])
                mT = apool.tile([P, FEAT], b16, tag="mT")
                nc.scalar.activation(out=mT[:], in_=pt[:], func=AF.Copy)
                return mT

            def stage_c(t, bt, mT):
                """Tail + relu + layer-1 fanout for tile t."""
                kind, g = tile_kind[t]
                # tail
                y = pa.tile([P, OUTW[0]], f32, tag="y")
                nc.tensor.matmul(y[:], lhsT=bt[:, 0:P], rhs=ws_ts[0][0][:],
                                 start=True, stop=False)
                nc.tensor.matmul(y[:], lhsT=bt[:, P:FEAT], rhs=ws_ts[0][1][:],
                                 start=False, stop=False)
                nc.tensor.matmul(y[:], lhsT=mT[:, 0:P], rhs=wn_ts[0][0][:],
                                 start=False, stop=False)
                nc.tensor.matmul(y[:], lhsT=mT[:, P:FEAT], rhs=wn_ts[0][1][:],
                                 start=False, stop=False)
                nc.tensor.matmul(y[:], lhsT=ones_t[0:1, :],
                                 rhs=bias_ts[0][0:1, :],
                                 start=False, stop=True)
                if t < N1_TILES:
                    o2 = h1head[t]
                else:
                    o2 = opool.tile([P, FEAT], b16, tag="o2")
                nc.scalar.activation(out=o2[:], in_=y[:], func=AF.Relu)
                # layer-1 fanout (mask rides the fused band stripe)
                fo = (1 + K[t]) * FEAT + ndma[t] * P
                if kind == "narrow":
                    st = first_t[g] == t
                    sp = last_t[g] == t
                    for f in range(2):
                        nc.tensor.matmul(
                            mean1[f][g][:],
                            lhsT=o2[:, f * P : (f + 1) * P],
                            rhs=bt[:, fo : fo + DHALF],
                            start=st, stop=sp,
                        )
                else:
                    for g2 in range(2):
                        st = first_t[g2] == t
                        sp = last_t[g2] == t
                        for f in range(2):
                            nc.tensor.matmul(
                                mean1[f][g2][:],
                                lhsT=o2[:, f * P : (f + 1) * P],
                                rhs=bt[:, fo + g2 * DHALF :
                                       fo + (g2 + 1) * DHALF],
                                start=st, stop=sp,
                            )

            # ===== layer 0 (pipeline: D+3 prefetch, then A, T, C) =====
            PF = 3
            bts = {}
            for t in tile_order[:PF]:
                bts[t] = stage_d(t)
            pipe = []
            for i, t in enumerate(tile_order):
                if i + PF < len(tile_order):
                    tn = tile_order[i + PF]
                    bts[tn] = stage_d(tn)
                mean0 = stage_a(t, bts[t])
                pipe.append([t, bts.pop(t), mean0, None])
                if len(pipe) >= 2:
                    e = pipe[-2]
                    e[3] = stage_t(e[0], e[1], e[2])
                if len(pipe) >= 3:
                    e = pipe.pop(0)
                    stage_c(e[0], e[1], e[3])
            for e in pipe:
                if e[3] is None:
                    e[3] = stage_t(e[0], e[1], e[2])
                stage_c(e[0], e[1], e[3])

            # ================= layer 1 =================
            meanT1 = [
                [cpool.tile([P, DHALF], b16, tag=f"meanT1_{f}_{g}",
                            name=f"meanT1_{f}_{g}")
                 for g in range(2)]
                for f in range(2)
            ]
            for f in range(2):
                for g in range(2):
                    nc.scalar.activation(out=meanT1[f][g][:],
                                         in_=mean1[f][g][:], func=AF.Copy)
            if debug:
                for u in range(N1_TILES):
                    nc.sync.dma_start(out=dbg_h1[u * P : (u + 1) * P, :],
                                      in_=h1head[u][:])
                for f in range(2):
                    for g in range(2):
                        nc.sync.dma_start(
                            out=dbg_m1[:, (2 * f + g) * DHALF :
                                       (2 * f + g + 1) * DHALF],
                            in_=meanT1[f][g][:])

            mean2 = pa.tile([P, FEAT], f32, tag="mean0", bufs=2)
            for u in range(N1_TILES):
                # transpose hd1 tile u
                pt = pa.tile([P, FEAT], b16, tag="pt")
                nc.tensor.transpose(out=pt[:, 0:P], in_=h1head[u][:, 0:P],
                                    identity=ident_t[:])
                nc.tensor.transpose(out=pt[:, P:FEAT],
                                    in_=h1head[u][:, P:FEAT],
                                    identity=ident_t[:])
                hdT = apool.tile([P, FEAT], b16, tag="mT")
                nc.scalar.activation(out=hdT[:], in_=pt[:], func=AF.Copy)
                y = pa.tile([P, OUTW[1]], f32, tag="y")
                g, j = u // 3, u % 3
                nc.tensor.matmul(y[:], lhsT=hdT[:, 0:P], rhs=ws_ts[1][0][:],
                                 start=True, stop=False)
                nc.tensor.matmul(y[:], lhsT=hdT[:, P:FEAT],
                                 rhs=ws_ts[1][1][:], start=False, stop=False)
                nc.tensor.matmul(y[:],
                                 lhsT=meanT1[0][g][:, j * P : (j + 1) * P],
                                 rhs=wn_ts[1][0][:], start=False, stop=False)
                nc.tensor.matmul(y[:],
                                 lhsT=meanT1[1][g][:, j * P : (j + 1) * P],
                                 rhs=wn_ts[1][1][:], start=False, stop=False)
                nc.tensor.matmul(y[:], lhsT=ones_t[0:1, :],
                                 rhs=bias_ts[1][0:1, :],
                                 start=False, stop=True)
                nc.scalar.activation(out=h2head[u][:], in_=y[:], func=AF.Relu)
                # layer-2 fanout: mean2 cols [0:64]=f0, [128:192]=f1
                # NOTE: start=True clears has_written for the WHOLE bank, so
                # only the very first matmul into this bank may carry it.
                for f in range(2):
                    nc.tensor.matmul(
                        mean2[:, f * P : f * P + SEEDS_PER_CORE],
                        lhsT=h2head[u][:, f * P : (f + 1) * P],
                        rhs=m2_t[:, u * SEEDS_PER_CORE : (u + 1) * SEEDS_PER_CORE],
                        start=(u == 0 and f == 0),
                        stop=(u == N1_TILES - 1),
                    )

            # ================= layer 2 =================
            mT2 = apool.tile([P, 2 * SEEDS_PER_CORE], b16, tag="mT2")
            nc.scalar.activation(out=mT2[:, 0:SEEDS_PER_CORE],
                                 in_=mean2[:, 0:SEEDS_PER_CORE], func=AF.Copy)
            nc.scalar.activation(out=mT2[:, SEEDS_PER_CORE:],
                                 in_=mean2[:, P : P + SEEDS_PER_CORE],
                                 func=AF.Copy)
            pt = pa.tile([P, FEAT], b16, tag="pt")
            nc.tensor.transpose(out=pt[:, 0:P], in_=h2head[0][:, 0:P],
                                identity=ident_t[:])
            nc.tensor.transpose(out=pt[:, P:FEAT], in_=h2head[0][:, P:FEAT],
                                identity=ident_t[:])
            hd2 = apool.tile([P, FEAT], b16, tag="mcopy")
            nc.scalar.activation(out=hd2[:], in_=pt[:], func=AF.Copy)
            y2 = pa.tile([P, OUTW[0]], f32, tag="y")
            S = SEEDS_PER_CORE
            nc.tensor.matmul(y2[0:S, 0:OUTW[2]], lhsT=hd2[:, 0:S],
                             rhs=ws_ts[2][0][:], start=True, stop=False)
            nc.tensor.matmul(y2[0:S, 0:OUTW[2]], lhsT=hd2[:, P : P + S],
                             rhs=ws_ts[2][1][:], start=False, stop=False)
            nc.tensor.matmul(y2[0:S, 0:OUTW[2]], lhsT=mT2[:, 0:S],
                             rhs=wn_ts[2][0][:], start=False, stop=False)
            nc.tensor.matmul(y2[0:S, 0:OUTW[2]], lhsT=mT2[:, S : 2 * S],
                             rhs=wn_ts[2][1][:], start=False, stop=False)
            nc.tensor.matmul(y2[0:S, 0:OUTW[2]], lhsT=ones_t[0:1, 0:S],
                             rhs=bias_ts[2][0:1, :], start=False, stop=True)
            o_f32 = apool.tile([P, OUTW[2]], f32, tag="ofin")
            nc.vector.tensor_copy(out=o_f32[0:S, :], in_=y2[0:S, 0:OUTW[2]])
            nc.sync.dma_start(out=out_d[:], in_=o_f32[0:S, :])
            if debug:
                for u in range(N1_TILES):
                    nc.sync.dma_start(out=dbg_h2[u * P : (u + 1) * P, :],
                                      in_=h2head[u][:])
                nc.sync.dma_start(out=dbg_m2[:], in_=mT2[:])

    nc.compile()

    bf16 = _bf16()
    eye16 = np.eye(P, dtype=bf16)
    colidx16 = np.broadcast_to(
        np.arange(P, dtype=np.float32), (P, P)
    ).astype(bf16)
    in_maps = []
    for c in range(NCORES):
        pc = meta["per_core"][c]
        m = dict(
            band=pc["band2"],
            mdst=pc["meta_dst"],
            mval=pc["meta_val"],
            m2=pc["m2"],
            ident=eye16,
            colidx=np.ascontiguousarray(colidx16),
            ones=np.ones((1, P), dtype=bf16),
        )
        for l in range(3):
            ws, wn, b = meta["weights"][l]
            m[f"ws{l}"] = np.ascontiguousarray(ws.astype(bf16))
            m[f"wn{l}"] = np.ascontiguousarray(wn.astype(bf16))
            m[f"bias{l}"] = np.ascontiguousarray(b[None, :].astype(bf16))
        in_maps.append(m)

    res = run_bass_kernel_spmd(
        nc, in_maps, core_ids=list(range(NCORES)), trace=trace
    )
    if debug:
        return [res.results[c] for c in range(NCORES)], res
    return [res.results[c]["out"] for c in range(NCORES)], res


def assemble(meta, outs):
    full = np.zeros((NUM_DST[2], OUTW[2]), np.float32)
    for c in range(NCORES):
        full[meta["blocks"][c]["seeds"]] = outs[c]
    return full


def kernel(**inputs) -> np.ndarray:
    meta = build_host(inputs)
    outs, _ = run_device(meta)
    return assemble(meta, outs)
